# revision 31
# baseline (speedup 1.0000x reference)
"""Fused cdist + 8-NN density kernel for Trainium2 (Bass/Tile), 8-core SPMD.

Problem: density-based point sampler. For each batch element (B=4) of
features (N=4096, C=60):
  d2[i,j] = sq[i] + sq[j] - 2*x@x.T;  d = sqrt(max(d2,0))
  density[i] = mean of the 8 smallest d in row i
  take indices of the M=819 largest densities (sorted desc), gather rows.

Sharding: one core per (batch, row-half): 8 cores, each computes the
density for 2048 rows against all 4096 columns, with the distance matrix
fused in PSUM/SBUF (never materialized in HBM).

Numerical strategy: the grading reference runs eagerly on the same
XLA-neuron backend; every fp op here replicates the reference's exact
operation order so that density matches bit-for-bit and the top-k index
selection is exact:
  - 2P via fp32 PE matmul with 2x pre-scaled stationary operand
    (scaling by 2 commutes with fp add exactly).
  - t1 = fl(sq_i + sq_j) via a K=2 PE matmul (single rounding).
  - nd2 = fl(2P - t1) via one DVE subtract (= exact negation of the
    reference's d2 = fl(t1 - 2P)).
  - top-8 via the DVE Max8 instruction on nd2 (selection by d2 gives the
    same d-multiset as the reference's selection by d).
  - sqrt only on the 8 selected values (scale=-1 folds the negation).
The tiny top-819 selection over 4096 densities + row gather runs on host.
"""

import numpy as np

B, N, C = 4, 4096, 60
K_NN = 8
SUB = 5
M_OUT = N // SUB  # 819
NCORES = 8
HALF = N // 2  # rows per core
NT_ROW = HALF // 128  # 16 row tiles per core
NT_ALL = N // 128  # 32 tiles for the sq pass

_cache = {}


def _build(debug=False):
    import concourse.mybir as mybir
    from concourse import bacc
    from concourse.bass import ts
    from concourse.masks import make_identity
    from concourse.tile import TileContext

    f32 = mybir.dt.float32
    AF = mybir.ActivationFunctionType
    ALU = mybir.AluOpType
    AX = mybir.AxisListType

    nc = bacc.Bacc()
    xt_h = nc.declare_dram_parameter("xt", [C, N], f32, isOutput=False)
    xn_h = nc.declare_dram_parameter("xn", [N, C], f32, isOutput=False)
    dens_h = nc.declare_dram_parameter("dens", [HALF], f32, isOutput=True)
    dbg_h = {}
    if debug:
        dbg_h["sqrow"] = nc.declare_dram_parameter("sqrow_dbg", [N], f32, isOutput=True)
        dbg_h["p2"] = nc.declare_dram_parameter("p2_dbg", [128, HALF], f32, isOutput=True)
        dbg_h["t1"] = nc.declare_dram_parameter("t1_dbg", [128, HALF], f32, isOutput=True)
        dbg_h["nd2"] = nc.declare_dram_parameter("nd2_dbg", [128, N], f32, isOutput=True)
        dbg_h["top8"] = nc.declare_dram_parameter("top8_dbg", [128, 8], f32, isOutput=True)
        dbg_h["d8"] = nc.declare_dram_parameter("d8_dbg", [128, 8], f32, isOutput=True)

    with TileContext(nc) as tc:
        with (
            tc.tile_pool(name="sb1", bufs=1) as sb1,
            tc.tile_pool(name="sbw", bufs=3) as sbw,
            tc.tile_pool(name="nd2p", bufs=2) as nd2p,
            tc.tile_pool(name="psA", bufs=2, space="PSUM") as psA,
        ):
            # ---- persistent buffers ----
            XT = sb1.tile([C, N], f32)  # x.T (rolled so this core's rows are 0..2047)
            nc.sync.dma_start(XT, xt_h[:, :])
            ident = sb1.tile([128, 128], f32)
            make_identity(nc, ident)
            # 2x stationary operand: 2 * x_rows.T
            # x2 scale on GpSimd: exact (x2 = exponent bump) and keeps the
            # bottleneck DVE free
            X2 = sb1.tile([C, HALF], f32)
            nc.gpsimd.tensor_scalar_mul(X2, XT[:, 0:HALF], 2.0)

            # ---- sq pass: sq[i] = fl-reduce(x[i,:]**2), mirrors eager mul+reduce ----
            # one DMA loads all of xn as [p, t, c] with row = 128t + p
            XN = sb1.tile([128, NT_ALL, C], f32)
            nc.sync.dma_start(XN, xn_h[:, :].rearrange("(t p) c -> p t c", p=128))
            X2N = sb1.tile([128, NT_ALL, C], f32)
            nc.vector.tensor_mul(X2N, XN, XN)
            SQA = sb1.tile([128, NT_ALL], f32)  # SQA[p, t] = sq[128t + p]
            nc.vector.tensor_reduce(SQA, X2N, axis=AX.X, op=ALU.add)
            pt = psA.tile([NT_ALL, 128], f32, tag="p2")
            nc.tensor.transpose(pt, SQA, ident)  # pt[t, p] = sq[128t + p]
            sqT = sb1.tile([NT_ALL, 128], f32)
            nc.scalar.activation(sqT, pt, AF.Copy)
            # flatten [32,128] -> a single row, then replicate across all 128
            # partitions (both cross-partition moves -> DMA)
            SQROW = sb1.tile([1, N], f32)
            nc.sync.dma_start(SQROW, sqT)
            # replicate sq_row across all 128 partitions via K=1 PE matmuls
            # with a ones column (1*sq is exact, no accumulation)
            ONES1 = sb1.tile([1, 128], f32)
            nc.gpsimd.memset(ONES1, 1.0)
            SQJ = sb1.tile([128, N], f32)
            for jb in range(N // 512):
                pb = psA.tile([128, 512], f32, tag="p2")
                nc.tensor.matmul(
                    pb,
                    lhsT=ONES1,
                    rhs=SQROW[:, ts(jb, 512)],
                    start=True,
                    stop=True,
                )
                nc.scalar.activation(SQJ[:, ts(jb, 512)], pb, AF.Copy)
            if debug:
                nc.sync.dma_start(dbg_h["sqrow"][:], SQROW[0:1, :])

            # ---- main loop: 16 row tiles x 4 quarter-chunks of 1024 cols ----
            for i in range(NT_ROW):
                nd2 = nd2p.tile([128, N], f32, tag="nd2")
                for h2 in range(2):
                    c0 = h2 * HALF
                    sl = slice(c0, c0 + HALF)
                    p2 = psA.tile([128, HALF], f32, tag="p2")
                    for jj in range(4):
                        nc.tensor.matmul(
                            p2[:, ts(jj, 512)],
                            lhsT=X2[:, ts(i, 128)],
                            rhs=XT[:, c0 + jj * 512 : c0 + (jj + 1) * 512],
                            start=True,
                            stop=True,
                        )
                    # t1 = fl(sq_i + sq_j) as a real elementwise add (the PE's
                    # fp32 accumulate rounds differently).  Relu == identity
                    # here (sq_i + sq_j > 0); Copy would reject an AP bias.
                    t1s = sbw.tile([128, HALF], f32, tag="t1s")
                    nc.scalar.activation(
                        t1s,
                        SQJ[:, sl],
                        AF.Relu,
                        bias=SQA[:, i : i + 1],
                        scale=1.0,
                    )
                    # nd2 = fl(2P - t1)  (== -d2 of the reference, bit-exact).
                    # Must be a single-rounding 2-input subtract -> DVE only
                    # (folding t1 into the PSUM accumulation rounds twice and
                    # perturbs the top-8 selection at the ulp level).
                    nc.vector.tensor_sub(nd2[:, sl], p2, t1s)
                    if debug and i == 0 and h2 == 0:
                        pd = sbw.tile([128, HALF], f32, tag="p2d")
                        nc.scalar.activation(pd, p2, AF.Copy)
                        nc.sync.dma_start(dbg_h["p2"][:, :], pd)
                        nc.sync.dma_start(dbg_h["t1"][:, :], t1s)
                    if debug and i == 0:
                        nc.sync.dma_start(dbg_h["nd2"][:, sl], nd2[:, sl])
                top8 = sbw.tile([128, 8], f32, tag="top8")
                nc.vector.max(top8, nd2)
                # clamp the diagonal's +/-eps to 0 (min/scale are exact ops,
                # so GpSimd is safe and keeps them off the bottleneck DVE)
                mins = sbw.tile([128, 8], f32, tag="mins")
                nc.gpsimd.tensor_scalar_min(mins, top8, 0.0)
                d8 = sbw.tile([128, 8], f32, tag="d8")
                nc.scalar.activation(d8, mins, AF.Sqrt, scale=-1.0)
                dsum = sbw.tile([128, 1], f32, tag="dsum")
                nc.vector.tensor_reduce(dsum, d8, axis=AX.X, op=ALU.add)
                dcol = sbw.tile([128, 1], f32, tag="dcol")
                nc.gpsimd.tensor_scalar_mul(dcol, dsum, 0.125)
                nc.sync.dma_start(dens_h[ts(i, 128)], dcol)
                if debug and i == 0:
                    nc.sync.dma_start(dbg_h["top8"][:, :], top8)
                    nc.sync.dma_start(dbg_h["d8"][:, :], d8)
    nc.finalize()  # runs the Bacc compile passes (sync-wait splitting etc.)
    return nc


def _get_nc(debug=False):
    key = ("nc", debug)
    if key not in _cache:
        _cache[key] = _build(debug)
    return _cache[key]


def _in_maps(feats):
    maps = []
    for core in range(NCORES):
        b, h = divmod(core, 2)
        if h == 0:
            xb = feats[b]
        else:
            xb = np.concatenate([feats[b][HALF:], feats[b][:HALF]], axis=0)
        xb = np.ascontiguousarray(xb)
        maps.append({"xn": xb, "xt": np.ascontiguousarray(xb.T)})
    return maps


def run_device(feats, debug=False, trace=False):
    """Run the 8-core SPMD kernel; returns (density (B,N), raw results obj)."""
    from concourse.bass_utils import run_bass_kernel_spmd

    nc = _get_nc(debug)
    res = run_bass_kernel_spmd(nc, _in_maps(feats), list(range(NCORES)), trace=trace)
    dens = np.empty((B, N), np.float32)
    for core in range(NCORES):
        b, h = divmod(core, 2)
        dens[b, h * HALF : (h + 1) * HALF] = res.results[core]["dens"]
    return dens, res


def kernel(features, pcd, ncam):
    feats = np.ascontiguousarray(np.asarray(features, dtype=np.float32))
    pcd = np.ascontiguousarray(np.asarray(pcd, dtype=np.float32))
    ncam = int(ncam)

    dens, _ = run_device(feats)

    n_per = N // ncam
    out_f = np.empty((B, M_OUT + ncam, C), np.float32)
    out_p = np.empty((B, M_OUT + ncam, 3), np.float32)
    out_c = np.empty((B, M_OUT), np.int32)
    for b in range(B):
        # matches jax.lax.top_k: largest first, ties -> lower index
        idx = np.argsort(-dens[b], kind="stable")[:M_OUT]
        out_f[b, :M_OUT] = feats[b][idx]
        out_p[b, :M_OUT] = pcd[b][idx]
        out_c[b] = (idx // n_per).astype(np.int32)
        out_f[b, M_OUT:] = (
            feats[b].reshape(ncam, n_per, C).mean(axis=1, dtype=np.float64).astype(np.float32)
        )
        out_p[b, M_OUT:] = (
            pcd[b].reshape(ncam, n_per, 3).mean(axis=1, dtype=np.float64).astype(np.float32)
        )
    return out_f, out_p, out_c


# revision 33
# speedup vs baseline: 1.0152x; 1.0152x over previous
"""Fused cdist + 8-NN density kernel for Trainium2 (Bass/Tile), 8-core SPMD.

Problem: density-based point sampler. For each batch element (B=4) of
features (N=4096, C=60):
  d2[i,j] = sq[i] + sq[j] - 2*x@x.T;  d = sqrt(max(d2,0))
  density[i] = mean of the 8 smallest d in row i
  take indices of the M=819 largest densities (sorted desc), gather rows.

Sharding: one core per (batch, row-half): 8 cores, each computes the
density for 2048 rows against all 4096 columns, with the distance matrix
fused in PSUM/SBUF (never materialized in HBM).

Numerical strategy: the grading reference runs eagerly on the same
XLA-neuron backend; every fp op here replicates the reference's exact
operation order so that density matches bit-for-bit and the top-k index
selection is exact:
  - 2P via fp32 PE matmul with 2x pre-scaled stationary operand
    (scaling by 2 commutes with fp add exactly).
  - t1 = fl(sq_i + sq_j) via a K=2 PE matmul (single rounding).
  - nd2 = fl(2P - t1) via one DVE subtract (= exact negation of the
    reference's d2 = fl(t1 - 2P)).
  - top-8 via the DVE Max8 instruction on nd2 (selection by d2 gives the
    same d-multiset as the reference's selection by d).
  - sqrt only on the 8 selected values (scale=-1 folds the negation).
The tiny top-819 selection over 4096 densities + row gather runs on host.
"""

import numpy as np

B, N, C = 4, 4096, 60
K_NN = 8
SUB = 5
M_OUT = N // SUB  # 819
NCORES = 8
HALF = N // 2  # rows per core
NT_ROW = HALF // 128  # 16 row tiles per core
NT_ALL = N // 128  # 32 tiles for the sq pass

_cache = {}


def _build(debug=False):
    import concourse.mybir as mybir
    from concourse import bacc
    from concourse.bass import ts
    from concourse.masks import make_identity
    from concourse.tile import TileContext

    f32 = mybir.dt.float32
    AF = mybir.ActivationFunctionType
    ALU = mybir.AluOpType
    AX = mybir.AxisListType

    nc = bacc.Bacc()
    xt_h = nc.declare_dram_parameter("xt", [C, N], f32, isOutput=False)
    xn_h = nc.declare_dram_parameter("xn", [N, C], f32, isOutput=False)
    dens_h = nc.declare_dram_parameter("dens", [HALF], f32, isOutput=True)
    dbg_h = {}
    if debug:
        dbg_h["sqrow"] = nc.declare_dram_parameter("sqrow_dbg", [N], f32, isOutput=True)
        dbg_h["p2"] = nc.declare_dram_parameter("p2_dbg", [128, HALF], f32, isOutput=True)
        dbg_h["t1"] = nc.declare_dram_parameter("t1_dbg", [128, HALF], f32, isOutput=True)
        dbg_h["nd2"] = nc.declare_dram_parameter("nd2_dbg", [128, N], f32, isOutput=True)
        dbg_h["top8"] = nc.declare_dram_parameter("top8_dbg", [128, 8], f32, isOutput=True)
        dbg_h["d8"] = nc.declare_dram_parameter("d8_dbg", [128, 8], f32, isOutput=True)

    with TileContext(nc) as tc:
        with (
            tc.tile_pool(name="sb1", bufs=1) as sb1,
            tc.tile_pool(name="sbw", bufs=4) as sbw,
            tc.tile_pool(name="nd2p", bufs=2) as nd2p,
            tc.tile_pool(name="psA", bufs=2, space="PSUM") as psA,
        ):
            # ---- sq pass (emitted first: it gates SQJ, the main-loop critical path): sq[i] = fl-reduce(x[i,:]**2), mirrors eager mul+reduce ----
            # one DMA loads all of xn as [p, t, c] with row = 128t + p
            XN = sb1.tile([128, NT_ALL, C], f32)
            nc.sync.dma_start(XN, xn_h[:, :].rearrange("(t p) c -> p t c", p=128))
            X2N = sb1.tile([128, NT_ALL, C], f32)
            nc.vector.tensor_mul(X2N, XN, XN)
            SQA = sb1.tile([128, NT_ALL], f32)  # SQA[p, t] = sq[128t + p]
            nc.vector.tensor_reduce(SQA, X2N, axis=AX.X, op=ALU.add)

            # ---- persistent buffers ----
            XT = sb1.tile([C, N], f32)  # x.T (rolled so this core's rows are 0..2047)
            nc.sync.dma_start(XT, xt_h[:, :])
            ident = sb1.tile([128, 128], f32)
            make_identity(nc, ident)
            # 2x stationary operand: 2 * x_rows.T
            # x2 scale on GpSimd: exact (x2 = exponent bump) and keeps the
            # bottleneck DVE free
            X2 = sb1.tile([C, HALF], f32)
            nc.gpsimd.tensor_scalar_mul(X2, XT[:, 0:HALF], 2.0)
            pt = psA.tile([NT_ALL, 128], f32, tag="p2")
            nc.tensor.transpose(pt, SQA, ident)  # pt[t, p] = sq[128t + p]
            sqT = sb1.tile([NT_ALL, 128], f32)
            nc.scalar.activation(sqT, pt, AF.Copy)
            # flatten [32,128] -> a single row, then replicate across all 128
            # partitions (both cross-partition moves -> DMA)
            SQROW = sb1.tile([1, N], f32)
            nc.sync.dma_start(SQROW, sqT)
            # replicate sq_row across all 128 partitions via K=1 PE matmuls
            # with a ones column (1*sq is exact, no accumulation)
            ONES1 = sb1.tile([1, 128], f32)
            nc.gpsimd.memset(ONES1, 1.0)
            SQJ = sb1.tile([128, N], f32)
            for jb in range(N // 512):
                pb = psA.tile([128, 512], f32, tag="p2")
                nc.tensor.matmul(
                    pb,
                    lhsT=ONES1,
                    rhs=SQROW[:, ts(jb, 512)],
                    start=True,
                    stop=True,
                )
                nc.scalar.activation(SQJ[:, ts(jb, 512)], pb, AF.Copy)
            if debug:
                nc.sync.dma_start(dbg_h["sqrow"][:], SQROW[0:1, :])

            # ---- main loop: 16 row tiles x 4 quarter-chunks of 1024 cols ----
            for i in range(NT_ROW):
                nd2 = nd2p.tile([128, N], f32, tag="nd2")
                for h2 in range(2):
                    c0 = h2 * HALF
                    sl = slice(c0, c0 + HALF)
                    p2 = psA.tile([128, HALF], f32, tag="p2")
                    for jj in range(4):
                        nc.tensor.matmul(
                            p2[:, ts(jj, 512)],
                            lhsT=X2[:, ts(i, 128)],
                            rhs=XT[:, c0 + jj * 512 : c0 + (jj + 1) * 512],
                            start=True,
                            stop=True,
                        )
                    # t1 = fl(sq_i + sq_j) as a real elementwise add (the PE's
                    # fp32 accumulate rounds differently).  Relu == identity
                    # here (sq_i + sq_j > 0); Copy would reject an AP bias.
                    t1s = sbw.tile([128, HALF], f32, tag="t1s")
                    nc.scalar.activation(
                        t1s,
                        SQJ[:, sl],
                        AF.Relu,
                        bias=SQA[:, i : i + 1],
                        scale=1.0,
                    )
                    # nd2 = fl(2P - t1)  (== -d2 of the reference, bit-exact).
                    # Must be a single-rounding 2-input subtract -> DVE only
                    # (folding t1 into the PSUM accumulation rounds twice and
                    # perturbs the top-8 selection at the ulp level).
                    nc.vector.tensor_sub(nd2[:, sl], p2, t1s)
                    if debug and i == 0 and h2 == 0:
                        pd = sbw.tile([128, HALF], f32, tag="p2d")
                        nc.scalar.activation(pd, p2, AF.Copy)
                        nc.sync.dma_start(dbg_h["p2"][:, :], pd)
                        nc.sync.dma_start(dbg_h["t1"][:, :], t1s)
                    if debug and i == 0:
                        nc.sync.dma_start(dbg_h["nd2"][:, sl], nd2[:, sl])
                top8 = sbw.tile([128, 8], f32, tag="top8")
                nc.vector.max(top8, nd2)
                # clamp the diagonal's +/-eps to 0 (min/scale are exact ops,
                # so GpSimd is safe and keeps them off the bottleneck DVE)
                mins = sbw.tile([128, 8], f32, tag="mins")
                nc.gpsimd.tensor_scalar_min(mins, top8, 0.0)
                d8 = sbw.tile([128, 8], f32, tag="d8")
                nc.scalar.activation(d8, mins, AF.Sqrt, scale=-1.0)
                dsum = sbw.tile([128, 1], f32, tag="dsum")
                nc.vector.tensor_reduce(dsum, d8, axis=AX.X, op=ALU.add)
                dcol = sbw.tile([128, 1], f32, tag="dcol")
                nc.gpsimd.tensor_scalar_mul(dcol, dsum, 0.125)
                nc.sync.dma_start(dens_h[ts(i, 128)], dcol)
                if debug and i == 0:
                    nc.sync.dma_start(dbg_h["top8"][:, :], top8)
                    nc.sync.dma_start(dbg_h["d8"][:, :], d8)
    nc.finalize()  # runs the Bacc compile passes (sync-wait splitting etc.)
    return nc


def _get_nc(debug=False):
    key = ("nc", debug)
    if key not in _cache:
        _cache[key] = _build(debug)
    return _cache[key]


def _in_maps(feats):
    maps = []
    for core in range(NCORES):
        b, h = divmod(core, 2)
        if h == 0:
            xb = feats[b]
        else:
            xb = np.concatenate([feats[b][HALF:], feats[b][:HALF]], axis=0)
        xb = np.ascontiguousarray(xb)
        maps.append({"xn": xb, "xt": np.ascontiguousarray(xb.T)})
    return maps


def run_device(feats, debug=False, trace=False):
    """Run the 8-core SPMD kernel; returns (density (B,N), raw results obj)."""
    from concourse.bass_utils import run_bass_kernel_spmd

    nc = _get_nc(debug)
    res = run_bass_kernel_spmd(nc, _in_maps(feats), list(range(NCORES)), trace=trace)
    dens = np.empty((B, N), np.float32)
    for core in range(NCORES):
        b, h = divmod(core, 2)
        dens[b, h * HALF : (h + 1) * HALF] = res.results[core]["dens"]
    return dens, res


def kernel(features, pcd, ncam):
    feats = np.ascontiguousarray(np.asarray(features, dtype=np.float32))
    pcd = np.ascontiguousarray(np.asarray(pcd, dtype=np.float32))
    ncam = int(ncam)

    dens, _ = run_device(feats)

    n_per = N // ncam
    out_f = np.empty((B, M_OUT + ncam, C), np.float32)
    out_p = np.empty((B, M_OUT + ncam, 3), np.float32)
    out_c = np.empty((B, M_OUT), np.int32)
    for b in range(B):
        # matches jax.lax.top_k: largest first, ties -> lower index
        idx = np.argsort(-dens[b], kind="stable")[:M_OUT]
        out_f[b, :M_OUT] = feats[b][idx]
        out_p[b, :M_OUT] = pcd[b][idx]
        out_c[b] = (idx // n_per).astype(np.int32)
        out_f[b, M_OUT:] = (
            feats[b].reshape(ncam, n_per, C).mean(axis=1, dtype=np.float64).astype(np.float32)
        )
        out_p[b, M_OUT:] = (
            pcd[b].reshape(ncam, n_per, 3).mean(axis=1, dtype=np.float64).astype(np.float32)
        )
    return out_f, out_p, out_c
